# revision 3
# baseline (speedup 1.0000x reference)
"""Trainium2 Bass kernel for nn_DiffusionBlock (anisotropic diffusion step).

Sharding: pure data-parallel over batch. 16 batches -> 8 cores x 2 batches;
each core processes 4 images (2 batches x 2 channels) of 768x768.

Math (same derivation as validated baseline, algebraically merged):
  grid 769x769 (i,j in 0..768), pu = edge-padded u (clamp at row/col 767)
  E = pu[i+1]-pu[i] ; Vs = pu[i+1]+pu[i]
  gp = Vs[j+1]-Vs[j] ; gm = E[j]+E[j+1] ; m = E[j]-E[j+1]
  with k4 = tau/8 folded into the a/b/c downcasts (Ab = k4*a etc., fp16):
    s12 = Ab*gp + Bb*gm ; s34 = Bb*gp + Cb*gm
    dsum = (Ab + Cb - 2*k4*|b|) * m ; dd = dsum[j+1]-dsum[j]
  acc = W1@s12_> - W1@s12 + W2@(s34_> + s34) + W2d@dd + I@pu
    W1 = S+I, W2 = S-I, W2d = (1-2a)(S-I), S = subdiagonal row-shift matrix
  out rows [t0, t0+126] stored fp16 from PSUM via GPSIMD copy.

Key layout trick: the 2 images of a group are interleaved in the minor dim
([128, W, 2] fp16) so every column shift is 4B-aligned -> all DVE
tensor_tensor ops run in 2x_1p mode. Engine split: ACT does f32->fp16
interleaving downcasts (+k4, and |b| via Abs); DVE 15 fp16 2x ops; PE 18
matmuls (incl +u via identity weight); GPSIMD evacuates PSUM + issues u
loads and stores; sync issues a/b/c loads. Row 767 from a small tail pass.
"""

import numpy as np
import ml_dtypes
from contextlib import ExitStack

import concourse.bass as bass
import concourse.mybir as mybir
import concourse.tile as tile
from concourse.bacc import Bacc
from concourse.bass_utils import run_bass_kernel_spmd

F32 = mybir.dt.float32
F16 = mybir.dt.float16
OP = mybir.AluOpType
AF = mybir.ActivationFunctionType

B, C, H, W = 16, 2, 768, 768
NCORES = 8
NIMG = 4          # images per core
IMGG = 2          # images per tile-group (interleaved in minor dim)
GW = 770          # u-grid width: cols 0..767 data, 768/769 replicate col 767
AW = 769          # a/b/c + s-field width (grid cols 0..768)
OW = 768
T0S = [0, 127, 254, 381, 508, 635, 640]


def _build(k4):
    nc = Bacc()
    u_d = nc.declare_dram_parameter("u", [NIMG, H, W], F32, isOutput=False)
    a_d = nc.declare_dram_parameter("a", [NIMG, H + 2, W + 2], F32, isOutput=False)
    b_d = nc.declare_dram_parameter("b", [NIMG, H + 2, W + 2], F32, isOutput=False)
    c_d = nc.declare_dram_parameter("c", [NIMG, H + 2, W + 2], F32, isOutput=False)
    s_d = nc.declare_dram_parameter("smat", [5, 128, 128], F16, isOutput=False)
    o_d = nc.declare_dram_parameter(
        "out", [NIMG // IMGG, H - 1, W, IMGG], F16, isOutput=True)
    o7_d = nc.declare_dram_parameter("out7", [NIMG, W], F16, isOutput=True)

    with tile.TileContext(nc) as tc, ExitStack() as ctx:
        consts = ctx.enter_context(tc.tile_pool(name="consts", bufs=1))
        loads = ctx.enter_context(tc.tile_pool(name="loads", bufs=2))
        half = ctx.enter_context(tc.tile_pool(name="half", bufs=2))
        scr = ctx.enter_context(tc.tile_pool(name="scr", bufs=2))
        outp = ctx.enter_context(tc.tile_pool(name="outp", bufs=2))
        psum = ctx.enter_context(
            tc.tile_pool(name="psum", bufs=2, space=bass.MemorySpace.PSUM))

        Wm = []
        for wi in range(5):
            wt = consts.tile([128, 128], F16, tag=f"w{wi}", name=f"w{wi}")
            nc.sync.dma_start(out=wt[:], in_=s_d[wi])
            Wm.append(wt[:])
        W1, W1n, W2, W2d, WI = Wm

        V = nc.vector
        GP = nc.gpsimd
        SC = nc.scalar

        for t0 in T0S:
            last = t0 == 640
            for g in range(NIMG // IMGG):
                ig0 = g * IMGG
                # ---- loads: u rows via GPSIMD SWDGE, a/b/c via sync HWDGE --
                PU = loads.tile([128, IMGG, GW], F32, tag="pu")
                PU2 = loads.tile([128, IMGG, GW], F32, tag="pu2")
                nd2 = min(128, H - (t0 + 1))  # 128 except last tile (127)
                src = u_d[ig0:ig0 + IMGG, t0:t0 + 128, :]
                GP.dma_start(out=PU[:, :, 0:W], in_=src.rearrange("i r c -> r i c"))
                src2 = u_d[ig0:ig0 + IMGG, t0 + 1:t0 + 1 + nd2, :]
                GP.dma_start(out=PU2[0:nd2, :, 0:W], in_=src2.rearrange("i r c -> r i c"))
                if nd2 < 128:
                    srcc = u_d[ig0:ig0 + IMGG, H - 1:H, :]
                    GP.dma_start(out=PU2[nd2:128, :, 0:W], in_=srcc.rearrange("i r c -> r i c"))
                A = loads.tile([128, IMGG, AW], F32, tag="a")
                Bt = loads.tile([128, IMGG, AW], F32, tag="b")
                Ct = loads.tile([128, IMGG, AW], F32, tag="c")
                for dram, buf in ((a_d, A), (b_d, Bt), (c_d, Ct)):
                    srcw = dram[ig0:ig0 + IMGG, 1 + t0:1 + t0 + 128, 1:W + 2]
                    nc.sync.dma_start(out=buf[:], in_=srcw.rearrange("i r c -> r i c"))
                # edge-replicate cols 768,769 := col 767 (f32, small)
                SC.copy(PU[:, :, W:W + 2], PU[:, :, W - 1:W].to_broadcast([128, IMGG, 2]))
                SC.copy(PU2[:, :, W:W + 2], PU2[:, :, W - 1:W].to_broadcast([128, IMGG, 2]))

                # ---- ACT: fp16 interleaved downcasts (k4 folded); 2k4|b| ----
                PUb = half.tile([128, GW, IMGG], F16, tag="pub")
                SC.copy(PUb[:].rearrange("p c i -> p i c"), PU[:])
                PU2b = half.tile([128, GW, IMGG], F16, tag="pu2b")
                SC.copy(PU2b[:].rearrange("p c i -> p i c"), PU2[:])
                Ab = half.tile([128, AW, IMGG], F16, tag="ab")
                SC.mul(Ab[:].rearrange("p c i -> p i c"), A[:], k4)
                Bb = half.tile([128, AW, IMGG], F16, tag="bb")
                SC.mul(Bb[:].rearrange("p c i -> p i c"), Bt[:], k4)
                Cb = half.tile([128, AW, IMGG], F16, tag="cb")
                SC.mul(Cb[:].rearrange("p c i -> p i c"), Ct[:], k4)
                absB2 = half.tile([128, AW, IMGG], F16, tag="absb")
                SC.activation(absB2[:].rearrange("p c i -> p i c"), Bt[:],
                              AF.Abs, scale=2.0 * k4)

                def S(tag, w=AW):
                    return scr.tile([128, w, IMGG], F16, tag=tag, name=tag)

                # ---- DVE stage (fp16 interleaved -> all 2x_1p) ----
                E = S("e", GW)
                V.tensor_sub(E[:], PU2b[:], PUb[:])
                Vs = S("vs", GW)
                V.tensor_add(Vs[:], PU2b[:], PUb[:])
                gp = S("gp")
                V.tensor_sub(gp[:], Vs[:, 1:GW, :], Vs[:, 0:AW, :])
                gm = S("gm")
                V.tensor_add(gm[:], E[:, 0:AW, :], E[:, 1:GW, :])
                m = S("m")
                V.tensor_sub(m[:], E[:, 0:AW, :], E[:, 1:GW, :])
                t1 = S("t1")
                V.tensor_mul(t1[:], Ab[:], gp[:])
                t2 = S("t2")
                V.tensor_mul(t2[:], Bb[:], gm[:])
                s12 = S("s12")
                V.tensor_add(s12[:], t1[:], t2[:])
                t3 = S("t1")
                V.tensor_mul(t3[:], Bb[:], gp[:])
                t4 = S("t2")
                V.tensor_mul(t4[:], Cb[:], gm[:])
                s34 = S("s34")
                V.tensor_add(s34[:], t3[:], t4[:])
                acb = S("t1")
                V.tensor_add(acb[:], Ab[:], Cb[:])
                acm = S("t2")
                V.tensor_sub(acm[:], acb[:], absB2[:])
                dsum = S("dsum")
                V.tensor_mul(dsum[:], acm[:], m[:])
                dd = S("dd", OW)
                V.tensor_sub(dd[:], dsum[:, 1:AW, :], dsum[:, 0:OW, :])

                # ---- PE: column stage + row shift + (+u), both images at once
                acc = psum.tile([128, OW, IMGG], F32, tag="acc")
                for c0 in (0, 256, 512):
                    cw = 256
                    terms = (
                        (W1, s12, 1), (W1n, s12, 0),
                        (W2, s34, 1), (W2, s34, 0),
                        (W2d, dd, 0), (WI, PUb, 0),
                    )
                    for ti, (wm, arr, sh) in enumerate(terms):
                        nc.tensor.matmul(
                            acc[:, c0:c0 + cw, :], wm,
                            arr[:, c0 + sh:c0 + sh + cw, :],
                            start=(ti == 0), stop=(ti == len(terms) - 1))

                # ---- evacuate PSUM -> fp16 (ACT), store via GPSIMD ----
                ot = outp.tile([128, OW, IMGG], F16, tag="ot")
                SC.copy(ot[0:127], acc[0:127])
                if not last:
                    p0, nr, r0 = 0, 127, t0
                else:
                    p0, nr, r0 = 122, 5, 762
                GP.dma_start(out=o_d[g, r0:r0 + nr], in_=ot[p0:p0 + nr])

        # ---- tail pass: output row 767, all 4 images on partitions 0..3 ----
        U7 = loads.tile([4, GW], F32, tag="a", name="u7")
        nc.sync.dma_start(out=U7[:, 0:W], in_=u_d[:, H - 1, :])
        SC.copy(U7[:, W:W + 2], U7[:, W - 1:W].to_broadcast([4, 2]))
        A7 = loads.tile([4, 2, AW], F32, tag="pu", name="a7")   # a' rows 767,768
        B7 = loads.tile([4, 2, AW], F32, tag="pu2", name="b7")
        nc.sync.dma_start(out=A7[:], in_=a_d[:, H:H + 2, 1:W + 2])
        nc.sync.dma_start(out=B7[:], in_=b_d[:, H:H + 2, 1:W + 2])
        D7 = scr.tile([4, AW], F32, tag="gp", name="d7t")
        V.tensor_sub(D7[:], U7[:, 1:GW], U7[:, 0:AW])
        aa = scr.tile([4, AW], F32, tag="gm", name="aa7t")   # a'[767] + a'[768]
        V.tensor_add(aa[:], A7[:, 0, :], A7[:, 1, :])
        bb = scr.tile([4, AW], F32, tag="m", name="bb7t")    # b'[768] - b'[767]
        V.tensor_sub(bb[:], B7[:, 1, :], B7[:, 0, :])
        sA = scr.tile([4, AW], F32, tag="t1", name="sa7t")   # s12[768]+s12[767]
        V.scalar_tensor_tensor(sA[:], aa[:], 2.0 * k4, D7[:], OP.mult, OP.mult)
        sB = scr.tile([4, AW], F32, tag="t2", name="sb7t")   # s34[768]-s34[767]
        V.scalar_tensor_tensor(sB[:], bb[:], 2.0 * k4, D7[:], OP.mult, OP.mult)
        tX = scr.tile([4, W], F32, tag="s12", name="tx7t")
        V.tensor_sub(tX[:], sA[:, 1:AW], sA[:, 0:W])
        tS = scr.tile([4, W], F32, tag="s34", name="ts7t")
        V.tensor_add(tS[:], sB[:, 1:AW], sB[:, 0:W])
        tZ = scr.tile([4, W], F32, tag="dsum", name="tz7t")
        V.tensor_add(tZ[:], tX[:], tS[:])
        o7 = scr.tile([4, W], F16, tag="dd", name="o77t")
        V.tensor_add(o7[:], tZ[:], U7[:, 0:W])
        GP.dma_start(out=o7_d[:], in_=o7[:])
    nc.finalize()
    return nc


def _smat(one_minus_2alpha):
    sh = np.zeros((128, 128), dtype=np.float32)
    for p in range(127):
        sh[p + 1, p] = 1.0   # sh[k, p] = 1 iff k = p+1  -> out[p] = in[p+1]
    ident = np.eye(128, dtype=np.float32)
    w1 = sh + ident
    w2 = sh - ident
    w2d = one_minus_2alpha * w2
    s = np.stack([w1, -w1, w2, w2d, ident])
    return s.astype(np.float16)


_cache = {}


def _get_nc(k4):
    if k4 not in _cache:
        _cache[k4] = _build(k4)
    return _cache[k4]


def kernel(u, a, b, c, grad_x1, grad_x2, grad_y1, grad_y2, alpha, tau):
    u = np.ascontiguousarray(np.asarray(u, dtype=np.float32))
    a = np.ascontiguousarray(np.asarray(a, dtype=np.float32))
    b = np.ascontiguousarray(np.asarray(b, dtype=np.float32))
    c = np.ascontiguousarray(np.asarray(c, dtype=np.float32))
    alpha_f = float(np.asarray(alpha))
    tau_f = float(np.asarray(tau))
    k4 = tau_f / 8.0

    nc = _get_nc(k4)
    smat = _smat(1.0 - 2.0 * alpha_f)

    bpc = B // NCORES  # batches per core
    in_maps = []
    for k in range(NCORES):
        sl = slice(bpc * k, bpc * (k + 1))
        in_maps.append({
            "u": np.ascontiguousarray(u[sl].reshape(NIMG, H, W)),
            "a": np.ascontiguousarray(a[sl].reshape(NIMG, H + 2, W + 2)),
            "b": np.ascontiguousarray(b[sl].reshape(NIMG, H + 2, W + 2)),
            "c": np.ascontiguousarray(c[sl].reshape(NIMG, H + 2, W + 2)),
            "smat": smat,
        })

    res = run_bass_kernel_spmd(nc, in_maps, list(range(NCORES)))
    out = np.empty((B, C, H, W), dtype=np.float32)
    for k in range(NCORES):
        r = np.asarray(res.results[k]["out"])      # [2, 767, 768, 2] fp16
        r7 = np.asarray(res.results[k]["out7"])    # [4, 768] fp16
        for g in range(bpc):
            bi = bpc * k + g
            out[bi, :, 0:H - 1, :] = np.transpose(
                r[g].astype(np.float32), (2, 0, 1))
            out[bi, :, H - 1, :] = r7[IMGG * g:IMGG * (g + 1)].astype(np.float32)
    return out


# revision 4
# speedup vs baseline: 1.2378x; 1.2378x over previous
"""Trainium2 Bass kernel for nn_DiffusionBlock (anisotropic diffusion step).

Sharding: pure data-parallel over batch. 16 batches -> 8 cores x 2 batches;
each core processes 4 images (2 batches x 2 channels) of 768x768.

Math (same derivation as validated baseline, algebraically merged):
  grid 769x769 (i,j in 0..768), pu = edge-padded u (clamp at row/col 767)
  E = pu[i+1]-pu[i] ; Vs = pu[i+1]+pu[i]
  gp = Vs[j+1]-Vs[j] ; gm = E[j]+E[j+1] ; m = E[j]-E[j+1]
  with k4 = tau/8 folded into the a/b/c downcasts (Ab = k4*a etc., fp16):
    s12 = Ab*gp + Bb*gm ; s34 = Bb*gp + Cb*gm
    dsum = (Ab + Cb - 2*k4*|b|) * m ; dd = dsum[j+1]-dsum[j]
  acc = W1@s12_> - W1@s12 + W2@(s34_> + s34) + W2d@dd + I@pu
    W1 = S+I, W2 = S-I, W2d = (1-2a)(S-I), S = subdiagonal row-shift matrix
  out rows [t0, t0+126] stored fp16; row 767 from a small tail pass.

Layout: host interleaves the 2 images of each batch in the minor dim
([..., 2] f32) so SBUF tiles are [128, W, 2]; every column shift is then
4B-aligned and all DVE tensor_tensor ops run in 2x_1p mode, DMA rows are
6KB single packets, and ACT downcasts are unit-stride on both sides.
Engine split per group: sync issues all 5 loads; ACT does 6 downcasts +
previous group's PSUM evacuation; DVE 10 fp16 2x ops; GPSIMD 5 fp16 ops +
previous group's store; PE 18 matmuls (incl +u via identity weight).
Evac/store are deferred one group so no engine queue blocks the pipeline.
"""

import numpy as np
import ml_dtypes
from contextlib import ExitStack

import concourse.bass as bass
import concourse.mybir as mybir
import concourse.tile as tile
from concourse.bacc import Bacc
from concourse.bass_utils import run_bass_kernel_spmd

F32 = mybir.dt.float32
F16 = mybir.dt.float16
OP = mybir.AluOpType
AF = mybir.ActivationFunctionType

B, C, H, W = 16, 2, 768, 768
NCORES = 8
NIMG = 4          # images per core
IMGG = 2          # images per tile-group (interleaved in minor dim)
NG = NIMG // IMGG
GW = 770          # u-grid width: cols 0..767 data, 768/769 replicate col 767
AW = 769          # a/b/c + s-field width (grid cols 0..768)
OW = 768
T0S = [0, 127, 254, 381, 508, 635, 640]


def _build(k4):
    nc = Bacc()
    u_d = nc.declare_dram_parameter("u", [NG, H, W, IMGG], F32, isOutput=False)
    a_d = nc.declare_dram_parameter("a", [NG, H + 2, W + 2, IMGG], F32, isOutput=False)
    b_d = nc.declare_dram_parameter("b", [NG, H + 2, W + 2, IMGG], F32, isOutput=False)
    c_d = nc.declare_dram_parameter("c", [NG, H + 2, W + 2, IMGG], F32, isOutput=False)
    u7_d = nc.declare_dram_parameter("u7", [NIMG, W], F32, isOutput=False)
    a7_d = nc.declare_dram_parameter("a7", [NIMG, 2, AW], F32, isOutput=False)
    b7_d = nc.declare_dram_parameter("b7", [NIMG, 2, AW], F32, isOutput=False)
    s_d = nc.declare_dram_parameter("smat", [5, 128, 128], F16, isOutput=False)
    o_d = nc.declare_dram_parameter("out", [NG, H - 1, W, IMGG], F16, isOutput=True)
    o7_d = nc.declare_dram_parameter("out7", [NIMG, W], F16, isOutput=True)

    with tile.TileContext(nc) as tc, ExitStack() as ctx:
        consts = ctx.enter_context(tc.tile_pool(name="consts", bufs=1))
        loads = ctx.enter_context(tc.tile_pool(name="loads", bufs=2))
        half = ctx.enter_context(tc.tile_pool(name="half", bufs=2))
        scr = ctx.enter_context(tc.tile_pool(name="scr", bufs=2))
        outp = ctx.enter_context(tc.tile_pool(name="outp", bufs=2))
        psum = ctx.enter_context(
            tc.tile_pool(name="psum", bufs=2, space=bass.MemorySpace.PSUM))

        Wm = []
        for wi in range(5):
            wt = consts.tile([128, 128], F16, tag=f"w{wi}", name=f"w{wi}")
            nc.sync.dma_start(out=wt[:], in_=s_d[wi])
            Wm.append(wt[:])
        W1, W1n, W2, W2d, WI = Wm

        V = nc.vector
        GP = nc.gpsimd
        SC = nc.scalar

        pend = None  # deferred (acc, g, r0, nr, p0) for evac+store

        def flush_pend():
            nonlocal pend
            if pend is None:
                return
            acc_p, g_p, r0_p, nr_p, p0_p = pend
            ot = outp.tile([128, OW, IMGG], F16, tag="ot", name="ot")
            SC.copy(ot[0:127], acc_p[0:127])
            GP.dma_start(out=o_d[g_p, r0_p:r0_p + nr_p], in_=ot[p0_p:p0_p + nr_p])
            pend = None

        for t0 in T0S:
            last = t0 == 640
            for g in range(NG):
                # ---- loads (all sync HWDGE; 6KB row packets) ----
                PU = loads.tile([128, GW, IMGG], F32, tag="pu", name="PU")
                PU2 = loads.tile([128, GW, IMGG], F32, tag="pu2", name="PU2")
                nd2 = min(128, H - (t0 + 1))  # 128 except last tile (127)
                nc.sync.dma_start(out=PU[:, 0:W, :], in_=u_d[g, t0:t0 + 128])
                nc.sync.dma_start(out=PU2[0:nd2, 0:W, :],
                                  in_=u_d[g, t0 + 1:t0 + 1 + nd2])
                if nd2 < 128:
                    nc.sync.dma_start(out=PU2[nd2:128, 0:W, :],
                                      in_=u_d[g, H - 1:H])
                A = loads.tile([128, AW, IMGG], F32, tag="a", name="A")
                Bt = loads.tile([128, AW, IMGG], F32, tag="b", name="Bt")
                Ct = loads.tile([128, AW, IMGG], F32, tag="c", name="Ct")
                for dram, buf in ((a_d, A), (b_d, Bt), (c_d, Ct)):
                    nc.sync.dma_start(out=buf[:],
                                      in_=dram[g, 1 + t0:1 + t0 + 128, 1:W + 2])

                # ---- ACT: pads, fp16 downcasts (k4 folded), 2k4|b| ----
                SC.copy(PU[:, W:W + 2, :],
                        PU[:, W - 1:W, :].to_broadcast([128, 2, IMGG]))
                SC.copy(PU2[:, W:W + 2, :],
                        PU2[:, W - 1:W, :].to_broadcast([128, 2, IMGG]))
                PUb = half.tile([128, GW, IMGG], F16, tag="pub", name="PUb")
                SC.copy(PUb[:], PU[:])
                PU2b = half.tile([128, GW, IMGG], F16, tag="pu2b", name="PU2b")
                SC.copy(PU2b[:], PU2[:])
                Ab = half.tile([128, AW, IMGG], F16, tag="ab", name="Ab")
                SC.mul(Ab[:], A[:], k4)
                Bb = half.tile([128, AW, IMGG], F16, tag="bb", name="Bb")
                SC.mul(Bb[:], Bt[:], k4)
                Cb = half.tile([128, AW, IMGG], F16, tag="cb", name="Cb")
                SC.mul(Cb[:], Ct[:], k4)
                absB2 = half.tile([128, AW, IMGG], F16, tag="absb", name="absB2")
                SC.activation(absB2[:], Bt[:], AF.Abs, scale=2.0 * k4)

                def S(tag, w=AW):
                    return scr.tile([128, w, IMGG], F16, tag=tag, name=tag)

                # ---- DVE stage (fp16 interleaved -> all 2x_1p) ----
                E = S("e", GW)
                V.tensor_sub(E[:], PU2b[:], PUb[:])
                Vs = S("vs", GW)
                V.tensor_add(Vs[:], PU2b[:], PUb[:])
                gp = S("gp")
                V.tensor_sub(gp[:], Vs[:, 1:GW, :], Vs[:, 0:AW, :])
                gm = S("gm")
                V.tensor_add(gm[:], E[:, 0:AW, :], E[:, 1:GW, :])
                m = S("m")
                V.tensor_sub(m[:], E[:, 0:AW, :], E[:, 1:GW, :])
                t1 = S("t1")
                V.tensor_mul(t1[:], Ab[:], gp[:])
                t2 = S("t2")
                V.tensor_mul(t2[:], Bb[:], gm[:])
                t3 = S("t3")
                V.tensor_mul(t3[:], Bb[:], gp[:])
                t4 = S("t4")
                V.tensor_mul(t4[:], Cb[:], gm[:])

                # ---- GPSIMD: combine ops (fp16) ----
                acb = S("acb")
                GP.tensor_add(acb[:], Ab[:], Cb[:])
                acm = S("acm")
                GP.tensor_sub(acm[:], acb[:], absB2[:])
                s12 = S("s12")
                GP.tensor_add(s12[:], t1[:], t2[:])
                s34 = S("s34")
                GP.tensor_add(s34[:], t3[:], t4[:])

                dsum = S("dsum")
                V.tensor_mul(dsum[:], acm[:], m[:])
                dd = S("dd", OW)
                GP.tensor_sub(dd[:], dsum[:, 1:AW, :], dsum[:, 0:OW, :])

                # ---- previous group's evac (ACT) + store (GPSIMD) ----
                flush_pend()

                # ---- PE: column stage + row shift + (+u), both images ----
                acc = psum.tile([128, OW, IMGG], F32, tag="acc", name="acc")
                for c0 in (0, 256, 512):
                    cw = 256
                    terms = (
                        (W1, s12, 1), (W1n, s12, 0),
                        (W2, s34, 1), (W2, s34, 0),
                        (W2d, dd, 0), (WI, PUb, 0),
                    )
                    for ti, (wm, arr, sh) in enumerate(terms):
                        nc.tensor.matmul(
                            acc[:, c0:c0 + cw, :], wm,
                            arr[:, c0 + sh:c0 + sh + cw, :],
                            start=(ti == 0), stop=(ti == len(terms) - 1))

                if not last:
                    pend = (acc, g, t0, 127, 0)
                else:
                    pend = (acc, g, 762, 5, 122)
        flush_pend()

        # ---- tail pass: output row 767, all 4 images on partitions 0..3 ----
        U7 = loads.tile([4, GW], F32, tag="a", name="u7t")
        nc.sync.dma_start(out=U7[:, 0:W], in_=u7_d[:])
        SC.copy(U7[:, W:W + 2], U7[:, W - 1:W].to_broadcast([4, 2]))
        A7 = loads.tile([4, 2, AW], F32, tag="pu", name="a7t")
        B7 = loads.tile([4, 2, AW], F32, tag="pu2", name="b7t")
        nc.sync.dma_start(out=A7[:], in_=a7_d[:])
        nc.sync.dma_start(out=B7[:], in_=b7_d[:])
        D7 = scr.tile([4, AW], F32, tag="gp", name="d7t")
        V.tensor_sub(D7[:], U7[:, 1:GW], U7[:, 0:AW])
        aa = scr.tile([4, AW], F32, tag="gm", name="aa7t")   # a'[767] + a'[768]
        V.tensor_add(aa[:], A7[:, 0, :], A7[:, 1, :])
        bb = scr.tile([4, AW], F32, tag="m", name="bb7t")    # b'[768] - b'[767]
        V.tensor_sub(bb[:], B7[:, 1, :], B7[:, 0, :])
        sA = scr.tile([4, AW], F32, tag="t1", name="sa7t")   # s12[768]+s12[767]
        V.scalar_tensor_tensor(sA[:], aa[:], 2.0 * k4, D7[:], OP.mult, OP.mult)
        sB = scr.tile([4, AW], F32, tag="t2", name="sb7t")   # s34[768]-s34[767]
        V.scalar_tensor_tensor(sB[:], bb[:], 2.0 * k4, D7[:], OP.mult, OP.mult)
        tX = scr.tile([4, W], F32, tag="s12", name="tx7t")
        V.tensor_sub(tX[:], sA[:, 1:AW], sA[:, 0:W])
        tS = scr.tile([4, W], F32, tag="s34", name="ts7t")
        V.tensor_add(tS[:], sB[:, 1:AW], sB[:, 0:W])
        tZ = scr.tile([4, W], F32, tag="dsum", name="tz7t")
        V.tensor_add(tZ[:], tX[:], tS[:])
        o7 = scr.tile([4, W], F16, tag="dd", name="o77t")
        V.tensor_add(o7[:], tZ[:], U7[:, 0:W])
        GP.dma_start(out=o7_d[:], in_=o7[:])
    nc.finalize()
    return nc


def _smat(one_minus_2alpha):
    sh = np.zeros((128, 128), dtype=np.float32)
    for p in range(127):
        sh[p + 1, p] = 1.0   # sh[k, p] = 1 iff k = p+1  -> out[p] = in[p+1]
    ident = np.eye(128, dtype=np.float32)
    w1 = sh + ident
    w2 = sh - ident
    w2d = one_minus_2alpha * w2
    s = np.stack([w1, -w1, w2, w2d, ident])
    return s.astype(np.float16)


_cache = {}


def _get_nc(k4):
    if k4 not in _cache:
        _cache[k4] = _build(k4)
    return _cache[k4]


def _prep_core(u4, a4, b4, c4):
    """Per-core host-side layout prep (pure transposes/slices, no math).

    u4..c4: [2, 2, H(+2), W(+2)] f32 (batch-local, channel, row, col).
    Returns dict of interleaved arrays: x[g, r, c, i] = x4[g, i, r, c].
    """
    il = lambda x: np.ascontiguousarray(np.transpose(x, (0, 2, 3, 1)))
    return {
        "u": il(u4), "a": il(a4), "b": il(b4), "c": il(c4),
        "u7": np.ascontiguousarray(u4[:, :, H - 1, :].reshape(NIMG, W)),
        "a7": np.ascontiguousarray(
            a4[:, :, H:H + 2, 1:W + 2].reshape(NIMG, 2, AW)),
        "b7": np.ascontiguousarray(
            b4[:, :, H:H + 2, 1:W + 2].reshape(NIMG, 2, AW)),
    }


def kernel(u, a, b, c, grad_x1, grad_x2, grad_y1, grad_y2, alpha, tau):
    u = np.ascontiguousarray(np.asarray(u, dtype=np.float32))
    a = np.ascontiguousarray(np.asarray(a, dtype=np.float32))
    b = np.ascontiguousarray(np.asarray(b, dtype=np.float32))
    c = np.ascontiguousarray(np.asarray(c, dtype=np.float32))
    alpha_f = float(np.asarray(alpha))
    tau_f = float(np.asarray(tau))
    k4 = tau_f / 8.0

    nc = _get_nc(k4)
    smat = _smat(1.0 - 2.0 * alpha_f)

    bpc = B // NCORES  # batches per core
    in_maps = []
    for k in range(NCORES):
        sl = slice(bpc * k, bpc * (k + 1))
        m = _prep_core(u[sl], a[sl], b[sl], c[sl])
        m["smat"] = smat
        in_maps.append(m)

    res = run_bass_kernel_spmd(nc, in_maps, list(range(NCORES)))
    out = np.empty((B, C, H, W), dtype=np.float32)
    for k in range(NCORES):
        r = np.asarray(res.results[k]["out"])      # [2, 767, 768, 2] fp16
        r7 = np.asarray(res.results[k]["out7"])    # [4, 768] fp16
        for g in range(bpc):
            bi = bpc * k + g
            out[bi, :, 0:H - 1, :] = np.transpose(
                r[g].astype(np.float32), (2, 0, 1))
            out[bi, :, H - 1, :] = r7[IMGG * g:IMGG * (g + 1)].astype(np.float32)
    return out


# revision 5
# speedup vs baseline: 1.5614x; 1.2615x over previous
"""Trainium2 Bass kernel for nn_DiffusionBlock (anisotropic diffusion step).

Sharding: pure data-parallel over batch. 16 batches -> 8 cores x 2 batches;
each core processes 4 images (2 batches x 2 channels) of 768x768.

Math (same derivation as validated baseline, algebraically merged):
  grid 769x769 (i,j in 0..768), pu = edge-padded u (clamp at row/col 767)
  E = pu[i+1]-pu[i] ; Vs = pu[i+1]+pu[i]
  gp = Vs[j+1]-Vs[j] ; gm = E[j]+E[j+1] ; m = E[j]-E[j+1]
  with k4 = tau/8 folded into the a/b/c downcasts (Ab = k4*a etc., fp16):
    s12 = Ab*gp + Bb*gm ; s34 = Bb*gp + Cb*gm
    dsum = (Ab + Cb - 2*k4*|b|) * m ; dd = dsum[j+1]-dsum[j]
  acc = W1@s12_> - W1@s12 + W2@(s34_> + s34) + W2d@dd + I@pu
    W1 = S+I, W2 = S-I, W2d = (1-2a)(S-I), S = subdiagonal row-shift matrix
  out rows [t0, t0+126] stored fp16; row 767 from a small tail pass.

Layout: host interleaves the 2 images of each batch in the minor dim
([..., 2] f32) so SBUF tiles are [128, W, 2]; every column shift is then
4B-aligned and all DVE tensor_tensor ops run in 2x_1p mode, DMA rows are
6KB single packets, and ACT downcasts are unit-stride on both sides.
Engine split per group: sync issues all 5 loads; ACT does 6 downcasts +
previous group's PSUM evacuation; DVE 10 fp16 2x ops; GPSIMD 5 fp16 ops +
previous group's store; PE 18 matmuls (incl +u via identity weight).
Evac/store are deferred one group so no engine queue blocks the pipeline.
"""

import numpy as np
import ml_dtypes
from contextlib import ExitStack

import concourse.bass as bass
import concourse.mybir as mybir
import concourse.tile as tile
from concourse.bacc import Bacc
from concourse.bass_utils import run_bass_kernel_spmd

F32 = mybir.dt.float32
F16 = mybir.dt.float16
OP = mybir.AluOpType
AF = mybir.ActivationFunctionType

B, C, H, W = 16, 2, 768, 768
NCORES = 8
NIMG = 4          # images per core
IMGG = 2          # images per tile-group (interleaved in minor dim)
NG = NIMG // IMGG
GW = 770          # u-grid width: cols 0..767 data, 768/769 replicate col 767
AW = 769          # a/b/c + s-field width (grid cols 0..768)
OW = 768
T0S = [0, 127, 254, 381, 508, 635, 640]


def _build(k4):
    nc = Bacc()
    u_d = nc.declare_dram_parameter("u", [NG, H, W, IMGG], F32, isOutput=False)
    a_d = nc.declare_dram_parameter("a", [NG, H + 2, W + 2, IMGG], F32, isOutput=False)
    b_d = nc.declare_dram_parameter("b", [NG, H + 2, W + 2, IMGG], F32, isOutput=False)
    c_d = nc.declare_dram_parameter("c", [NG, H + 2, W + 2, IMGG], F32, isOutput=False)
    u7_d = nc.declare_dram_parameter("u7", [NIMG, W], F32, isOutput=False)
    a7_d = nc.declare_dram_parameter("a7", [NIMG, 2, AW], F32, isOutput=False)
    b7_d = nc.declare_dram_parameter("b7", [NIMG, 2, AW], F32, isOutput=False)
    s_d = nc.declare_dram_parameter("smat", [5, 128, 128], F16, isOutput=False)
    o_d = nc.declare_dram_parameter("out", [NG, H - 1, W, IMGG], F16, isOutput=True)
    o7_d = nc.declare_dram_parameter("out7", [NIMG, W], F16, isOutput=True)

    with tile.TileContext(nc) as tc, ExitStack() as ctx:
        consts = ctx.enter_context(tc.tile_pool(name="consts", bufs=1))
        loads = ctx.enter_context(tc.tile_pool(name="loads", bufs=2))
        half = ctx.enter_context(tc.tile_pool(name="half", bufs=2))
        scr = ctx.enter_context(tc.tile_pool(name="scr", bufs=2))
        outp = ctx.enter_context(tc.tile_pool(name="outp", bufs=2))
        psum = ctx.enter_context(
            tc.tile_pool(name="psum", bufs=2, space=bass.MemorySpace.PSUM))

        Wm = []
        for wi in range(5):
            wt = consts.tile([128, 128], F16, tag=f"w{wi}", name=f"w{wi}")
            nc.sync.dma_start(out=wt[:], in_=s_d[wi])
            Wm.append(wt[:])
        W1, W1n, W2, W2d, WI = Wm

        V = nc.vector
        GP = nc.gpsimd
        SC = nc.scalar

        pend = None  # deferred (acc, g, r0, nr, p0) for evac+store

        def flush_pend():
            nonlocal pend
            if pend is None:
                return
            acc_p, g_p, r0_p, nr_p, p0_p = pend
            ot = outp.tile([128, OW, IMGG], F16, tag="ot", name="ot")
            SC.copy(ot[0:127], acc_p[0:127])
            GP.dma_start(out=o_d[g_p, r0_p:r0_p + nr_p], in_=ot[p0_p:p0_p + nr_p])
            pend = None

        for t0 in T0S:
            last = t0 == 640
            for g in range(NG):
                # ---- loads (all sync HWDGE; 6KB row packets) ----
                PU = loads.tile([128, GW, IMGG], F32, tag="pu", name="PU")
                PU2 = loads.tile([128, GW, IMGG], F32, tag="pu2", name="PU2")
                nd2 = min(128, H - (t0 + 1))  # 128 except last tile (127)
                nc.sync.dma_start(out=PU[:, 0:W, :], in_=u_d[g, t0:t0 + 128])
                nc.sync.dma_start(out=PU2[0:nd2, 0:W, :],
                                  in_=u_d[g, t0 + 1:t0 + 1 + nd2])
                if nd2 < 128:
                    nc.sync.dma_start(out=PU2[nd2:128, 0:W, :],
                                      in_=u_d[g, H - 1:H])
                A = loads.tile([128, AW, IMGG], F32, tag="a", name="A")
                Bt = loads.tile([128, AW, IMGG], F32, tag="b", name="Bt")
                Ct = loads.tile([128, AW, IMGG], F32, tag="c", name="Ct")
                for dram, buf in ((a_d, A), (b_d, Bt), (c_d, Ct)):
                    nc.sync.dma_start(out=buf[:],
                                      in_=dram[g, 1 + t0:1 + t0 + 128, 1:W + 2])

                # ---- ACT: pads, fp16 downcasts (k4 folded), 2k4|b| ----
                SC.copy(PU[:, W:W + 2, :],
                        PU[:, W - 1:W, :].to_broadcast([128, 2, IMGG]))
                SC.copy(PU2[:, W:W + 2, :],
                        PU2[:, W - 1:W, :].to_broadcast([128, 2, IMGG]))
                PUb = half.tile([128, GW, IMGG], F16, tag="pub", name="PUb")
                SC.copy(PUb[:], PU[:])
                PU2b = half.tile([128, GW, IMGG], F16, tag="pu2b", name="PU2b")
                SC.copy(PU2b[:], PU2[:])
                Ab = half.tile([128, AW, IMGG], F16, tag="ab", name="Ab")
                SC.mul(Ab[:], A[:], k4)
                Bb = half.tile([128, AW, IMGG], F16, tag="bb", name="Bb")
                SC.mul(Bb[:], Bt[:], k4)
                Cb = half.tile([128, AW, IMGG], F16, tag="cb", name="Cb")
                SC.mul(Cb[:], Ct[:], k4)
                absB2 = half.tile([128, AW, IMGG], F16, tag="absb", name="absB2")
                SC.activation(absB2[:], Bt[:], AF.Abs, scale=2.0 * k4)

                def S(tag, w=AW):
                    return scr.tile([128, w, IMGG], F16, tag=tag, name=tag)

                # ---- DVE stage (fp16 interleaved -> all 2x_1p) ----
                E = S("e", GW)
                V.tensor_sub(E[:], PU2b[:], PUb[:])
                Vs = S("vs", GW)
                V.tensor_add(Vs[:], PU2b[:], PUb[:])
                gp = S("gp")
                V.tensor_sub(gp[:], Vs[:, 1:GW, :], Vs[:, 0:AW, :])
                gm = S("gm")
                V.tensor_add(gm[:], E[:, 0:AW, :], E[:, 1:GW, :])
                m = S("m")
                V.tensor_sub(m[:], E[:, 0:AW, :], E[:, 1:GW, :])
                t1 = S("t1")
                V.tensor_mul(t1[:], Ab[:], gp[:])
                t2 = S("t2")
                V.tensor_mul(t2[:], Bb[:], gm[:])
                t3 = S("t3")
                V.tensor_mul(t3[:], Bb[:], gp[:])
                t4 = S("t4")
                V.tensor_mul(t4[:], Cb[:], gm[:])

                acb = S("acb")
                V.tensor_add(acb[:], Ab[:], Cb[:])
                acm = S("acm")
                V.tensor_sub(acm[:], acb[:], absB2[:])
                s12 = S("s12")
                V.tensor_add(s12[:], t1[:], t2[:])
                s34 = S("s34")
                V.tensor_add(s34[:], t3[:], t4[:])
                dsum = S("dsum")
                V.tensor_mul(dsum[:], acm[:], m[:])
                dd = S("dd", OW)
                V.tensor_sub(dd[:], dsum[:, 1:AW, :], dsum[:, 0:OW, :])

                # ---- previous group's evac (ACT) + store (GPSIMD) ----
                flush_pend()

                # ---- PE: column stage + row shift + (+u), both images ----
                acc = psum.tile([128, OW, IMGG], F32, tag="acc", name="acc")
                for c0 in (0, 256, 512):
                    cw = 256
                    terms = (
                        (W1, s12, 1), (W1n, s12, 0),
                        (W2, s34, 1), (W2, s34, 0),
                        (W2d, dd, 0), (WI, PUb, 0),
                    )
                    for ti, (wm, arr, sh) in enumerate(terms):
                        nc.tensor.matmul(
                            acc[:, c0:c0 + cw, :], wm,
                            arr[:, c0 + sh:c0 + sh + cw, :],
                            start=(ti == 0), stop=(ti == len(terms) - 1))

                if not last:
                    pend = (acc, g, t0, 127, 0)
                else:
                    pend = (acc, g, 762, 5, 122)
        flush_pend()

        # ---- tail pass: output row 767, all 4 images on partitions 0..3 ----
        U7 = loads.tile([4, GW], F32, tag="a", name="u7t")
        nc.sync.dma_start(out=U7[:, 0:W], in_=u7_d[:])
        SC.copy(U7[:, W:W + 2], U7[:, W - 1:W].to_broadcast([4, 2]))
        A7 = loads.tile([4, 2, AW], F32, tag="pu", name="a7t")
        B7 = loads.tile([4, 2, AW], F32, tag="pu2", name="b7t")
        nc.sync.dma_start(out=A7[:], in_=a7_d[:])
        nc.sync.dma_start(out=B7[:], in_=b7_d[:])
        D7 = scr.tile([4, AW], F32, tag="gp", name="d7t")
        V.tensor_sub(D7[:], U7[:, 1:GW], U7[:, 0:AW])
        aa = scr.tile([4, AW], F32, tag="gm", name="aa7t")   # a'[767] + a'[768]
        V.tensor_add(aa[:], A7[:, 0, :], A7[:, 1, :])
        bb = scr.tile([4, AW], F32, tag="m", name="bb7t")    # b'[768] - b'[767]
        V.tensor_sub(bb[:], B7[:, 1, :], B7[:, 0, :])
        sA = scr.tile([4, AW], F32, tag="t1", name="sa7t")   # s12[768]+s12[767]
        V.scalar_tensor_tensor(sA[:], aa[:], 2.0 * k4, D7[:], OP.mult, OP.mult)
        sB = scr.tile([4, AW], F32, tag="t2", name="sb7t")   # s34[768]-s34[767]
        V.scalar_tensor_tensor(sB[:], bb[:], 2.0 * k4, D7[:], OP.mult, OP.mult)
        tX = scr.tile([4, W], F32, tag="s12", name="tx7t")
        V.tensor_sub(tX[:], sA[:, 1:AW], sA[:, 0:W])
        tS = scr.tile([4, W], F32, tag="s34", name="ts7t")
        V.tensor_add(tS[:], sB[:, 1:AW], sB[:, 0:W])
        tZ = scr.tile([4, W], F32, tag="dsum", name="tz7t")
        V.tensor_add(tZ[:], tX[:], tS[:])
        o7 = scr.tile([4, W], F16, tag="dd", name="o77t")
        V.tensor_add(o7[:], tZ[:], U7[:, 0:W])
        GP.dma_start(out=o7_d[:], in_=o7[:])
    nc.finalize()
    return nc


def _smat(one_minus_2alpha):
    sh = np.zeros((128, 128), dtype=np.float32)
    for p in range(127):
        sh[p + 1, p] = 1.0   # sh[k, p] = 1 iff k = p+1  -> out[p] = in[p+1]
    ident = np.eye(128, dtype=np.float32)
    w1 = sh + ident
    w2 = sh - ident
    w2d = one_minus_2alpha * w2
    s = np.stack([w1, -w1, w2, w2d, ident])
    return s.astype(np.float16)


_cache = {}


def _get_nc(k4):
    if k4 not in _cache:
        _cache[k4] = _build(k4)
    return _cache[k4]


def _prep_core(u4, a4, b4, c4):
    """Per-core host-side layout prep (pure transposes/slices, no math).

    u4..c4: [2, 2, H(+2), W(+2)] f32 (batch-local, channel, row, col).
    Returns dict of interleaved arrays: x[g, r, c, i] = x4[g, i, r, c].
    """
    il = lambda x: np.ascontiguousarray(np.transpose(x, (0, 2, 3, 1)))
    return {
        "u": il(u4), "a": il(a4), "b": il(b4), "c": il(c4),
        "u7": np.ascontiguousarray(u4[:, :, H - 1, :].reshape(NIMG, W)),
        "a7": np.ascontiguousarray(
            a4[:, :, H:H + 2, 1:W + 2].reshape(NIMG, 2, AW)),
        "b7": np.ascontiguousarray(
            b4[:, :, H:H + 2, 1:W + 2].reshape(NIMG, 2, AW)),
    }


def kernel(u, a, b, c, grad_x1, grad_x2, grad_y1, grad_y2, alpha, tau):
    u = np.ascontiguousarray(np.asarray(u, dtype=np.float32))
    a = np.ascontiguousarray(np.asarray(a, dtype=np.float32))
    b = np.ascontiguousarray(np.asarray(b, dtype=np.float32))
    c = np.ascontiguousarray(np.asarray(c, dtype=np.float32))
    alpha_f = float(np.asarray(alpha))
    tau_f = float(np.asarray(tau))
    k4 = tau_f / 8.0

    nc = _get_nc(k4)
    smat = _smat(1.0 - 2.0 * alpha_f)

    bpc = B // NCORES  # batches per core
    in_maps = []
    for k in range(NCORES):
        sl = slice(bpc * k, bpc * (k + 1))
        m = _prep_core(u[sl], a[sl], b[sl], c[sl])
        m["smat"] = smat
        in_maps.append(m)

    res = run_bass_kernel_spmd(nc, in_maps, list(range(NCORES)))
    out = np.empty((B, C, H, W), dtype=np.float32)
    for k in range(NCORES):
        r = np.asarray(res.results[k]["out"])      # [2, 767, 768, 2] fp16
        r7 = np.asarray(res.results[k]["out7"])    # [4, 768] fp16
        for g in range(bpc):
            bi = bpc * k + g
            out[bi, :, 0:H - 1, :] = np.transpose(
                r[g].astype(np.float32), (2, 0, 1))
            out[bi, :, H - 1, :] = r7[IMGG * g:IMGG * (g + 1)].astype(np.float32)
    return out
